# revision 1
# baseline (speedup 1.0000x reference)
"""3-layer GCN + global mean/max pool + linear classifier on 8 Trainium2 NeuronCores.

Strategy (node-parallel, NOT edge-parallel):
  * Aggregate-first algebra: Agg(x @ W) == Agg(x) @ W, with the symmetric
    normalization folded into pre-scaled features  xt = dinv * x  so the
    per-edge multiply disappears:
        out[v] = dinv[v] * ( sum_{e: dst=v} xt[src_e] + xt[v] ) @ W + b
  * Shard dst nodes across 8 cores at graph-aligned boundaries (64 graphs /
    core).  Each core aggregates only its own ~12.5K dst rows (~400K edges),
    gathering source rows from a replicated feature table via indirect DMA
    (128 rows x 512B per "slot", batched ~1MB per DMA instruction).
  * Per-core dst nodes are sorted by in-degree so fixed-slot tiles have
    almost no padding.  All per-core variation (indices, scales, graph ids)
    is input data => one SPMD program for all cores.
  * After layers 1-2 each core's output shard is AllGather'd (in chunks, to
    overlap with compute) into the next layer's gather table.  Layer 3 output
    stays local: pooling only needs the core's own 64 graphs.
  * Sum-pool via PE matmul with a data-driven graph-indicator matrix; max-pool
    via a second small indirect gather pass over the local h3 scratch.

kernel(**inputs) takes the full unsharded inputs and returns the full
[512, 2] output.
"""

import os
import sys

import numpy as np

sys.path.insert(0, "/opt/trn_rl_repo")

N_CORES = 8
GPAIR = 8  # pair-slots per main gather DMA (gather buf = [128, GPAIR*256])
POOL_G = 16  # slots per pooling gather DMA
N_CHUNKS = 4  # allgather chunks per layer
TABLE_DT = os.environ.get("GCN_TABLE_DT", "float32")  # float32 | bfloat16

LAST_RESULTS = None  # BassKernelResults of the most recent run (for test.py)


# --------------------------------------------------------------------------
# host-side graph preprocessing
# --------------------------------------------------------------------------
def _host_prep(x, edge_index, batch, n_cores, n_graphs, n_chunks, shared_tables=False):
    """Compute all per-core index/scale arrays and the layout metadata.

    shared_tables: layer-2/3 gather tables are Shared-address-space DRAM
    written by a single mesh AllGather (needs n_cores > 4).  The zero row is
    then a pad row inside some core's stripe (pad rows compute to exact 0).
    """
    if shared_tables:
        n_chunks = 1
    x = np.asarray(x, dtype=np.float32)
    edge_index = np.asarray(edge_index)
    batch = np.asarray(batch).astype(np.int64)
    N, D = x.shape
    E = edge_index.shape[1]
    gpc = n_graphs // n_cores

    src = edge_index[0].astype(np.int64)
    dst = edge_index[1].astype(np.int64)
    indeg = np.bincount(dst, minlength=N).astype(np.int64)
    deg = (indeg + 1).astype(np.float64)  # + self loop
    dinv = (1.0 / np.sqrt(deg)).astype(np.float32)

    xt = x * dinv[:, None]
    xt = np.concatenate([xt, np.zeros((1, D), np.float32)], 0)  # zero row at N

    # graph boundaries (batch is sorted)
    gstart = np.searchsorted(batch, np.arange(n_graphs + 1)).astype(np.int64)
    B = gstart[np.arange(n_cores + 1) * gpc]  # core node boundaries

    # per-core degree-sorted permutation of owned nodes
    perms = []
    for i in range(n_cores):
        nodes = np.arange(B[i], B[i + 1])
        perms.append(nodes[np.argsort(-indeg[nodes], kind="stable")])
    n_own = np.array([len(p) for p in perms])
    n_tiles = int(np.ceil((n_own.max() + (1 if shared_tables else 0)) / 128))
    n_tiles += n_tiles % 2  # even, for pair supertiles
    S_rows = n_tiles * 128
    n_super = n_tiles // 2

    # localpos[v] = position of node v within its core's permuted layout
    localpos = np.zeros(N, dtype=np.int64)
    for p in perms:
        localpos[p] = np.arange(len(p))

    # slots per supertile: 1 (self) + max in-degree among its 256 nodes,
    # maxed across cores (static SPMD program).
    slots_u = np.ones(n_super, dtype=np.int64)
    for i in range(n_cores):
        dg = np.zeros(S_rows, dtype=np.int64)
        dg[: n_own[i]] = indeg[perms[i]]
        m = dg.reshape(n_super, 256).max(1)
        slots_u = np.maximum(slots_u, 1 + m)
    M = int(slots_u.max())
    col_of_super = np.concatenate([[0], np.cumsum(2 * slots_u)]).astype(np.int64)
    S_cols = int(col_of_super[-1])

    # edge lists sorted by dst, with per-dst rank
    eo = np.argsort(dst, kind="stable")
    ds, ss = dst[eo], src[eo]
    eptr = np.searchsorted(ds, np.arange(N + 1))
    erank = np.arange(E, dtype=np.int64) - eptr[ds]

    # chunk layout over supertiles
    chunk_supers = np.array_split(np.arange(n_super), n_chunks)
    chunk_u0 = [int(cs[0]) if len(cs) else 0 for cs in chunk_supers]
    chunk_rows = [len(cs) * 256 for cs in chunk_supers]
    chunk_off = np.concatenate([[0], np.cumsum(np.array(chunk_rows) * n_cores)])
    chunk_of_super = np.zeros(n_super, dtype=np.int64)
    for c, cs in enumerate(chunk_supers):
        chunk_of_super[cs] = c

    # table position of each node for layers 2/3
    chunk_rows_arr = np.array(chunk_rows, dtype=np.int64)
    chunk_base = np.array([chunk_u0[cc] * 256 for cc in range(n_chunks)], dtype=np.int64)

    def pos_of(core, r):
        c = chunk_of_super[np.asarray(r) // 256]
        return chunk_off[c] + core * chunk_rows_arr[c] + (np.asarray(r) - chunk_base[c])

    if shared_tables:
        # zero row = first pad row of the core with the most padding
        k0 = int(np.argmin(n_own))
        assert n_own[k0] < S_rows, "no pad rows available for the zero row"
        Z = int(pos_of(k0, n_own[k0]))
        T_rows = int(chunk_off[-1])
    else:
        Z = int(chunk_off[-1])  # extra zero row appended past all stripes
        T_rows = Z + 1

    pos23 = np.full(N + 1, Z, dtype=np.int64)
    for i in range(n_cores):
        r = np.arange(n_own[i])
        pos23[perms[i]] = pos_of(i, r)

    per_core = []
    for i in range(n_cores):
        # node-id grid [S_rows, M]; -1 = padding
        grid = np.full((S_rows, M), -1, dtype=np.int64)
        grid[: n_own[i], 0] = perms[i]
        m = (ds >= B[i]) & (ds < B[i + 1])
        grid[localpos[ds[m]], 1 + erank[m]] = ss[m]

        def grid_to_idx(posmap, padpos):
            cols = []
            for u in range(n_super):
                su = int(slots_u[u])
                sub = grid[u * 256 : (u + 1) * 256, :su]
                p = np.where(sub < 0, padpos, posmap[np.clip(sub, 0, None)])
                cols.append(
                    p.reshape(2, 128, su).transpose(1, 2, 0).reshape(128, 2 * su)
                )
            return np.ascontiguousarray(
                np.concatenate(cols, axis=1).astype(np.int32)
            )

        idx1 = grid_to_idx(np.arange(N + 1, dtype=np.int64), N)
        idx23 = grid_to_idx(pos23, Z)

        # per-tile scale columns (perm order, pads = 0)
        dv = np.zeros(S_rows, dtype=np.float32)
        dv[: n_own[i]] = dinv[perms[i]]
        d2c = np.ascontiguousarray((dv * dv).reshape(n_tiles, 128).T)
        d1c = np.ascontiguousarray(dv.reshape(n_tiles, 128).T)
        dvr = dv.reshape(1, S_rows)

        # graph id (local) of each perm row; pads = -1
        gl = np.full(S_rows, -1.0, dtype=np.float32)
        gl[: n_own[i]] = (batch[perms[i]] - i * gpc).astype(np.float32)
        gid = np.ascontiguousarray(gl.reshape(n_tiles, 128).T)

        per_core.append(
            dict(idx1=idx1, idx23=idx23, d2c=d2c, d1c=d1c, dvr=dvr, gid=gid)
        )

    # pooling: P_slots = max graph size (global); pool idx [gpc, P_slots]
    cnt_all = np.diff(gstart)
    P_slots = int(cnt_all.max())
    P_slots = -(-P_slots // POOL_G) * POOL_G  # round up to POOL_G
    for i in range(n_cores):
        pidx = np.full((gpc, P_slots), S_rows, dtype=np.int32)  # -inf row
        cnt = np.zeros(gpc, dtype=np.int64)
        for g in range(gpc):
            s, e = gstart[i * gpc + g], gstart[i * gpc + g + 1]
            cnt[g] = e - s
            pidx[g, : e - s] = localpos[np.arange(s, e)]
        per_core[i]["pidx"] = pidx
        ic = np.where(cnt > 0, 1.0 / np.maximum(cnt, 1), 0.0).astype(np.float32)
        per_core[i]["icnt"] = np.ascontiguousarray(
            np.broadcast_to(ic[None, :], (2, gpc))
        )

    return dict(
        N=N,
        D=D,
        gpc=gpc,
        n_cores=n_cores,
        n_tiles=n_tiles,
        n_super=n_super,
        S_rows=S_rows,
        S_cols=S_cols,
        slots_u=slots_u,
        col_of_super=col_of_super,
        chunk_supers=chunk_supers,
        chunk_u0=chunk_u0,
        chunk_rows=chunk_rows,
        chunk_off=chunk_off,
        Z=Z,
        T_rows=T_rows,
        P_slots=P_slots,
        xt=xt,
        per_core=per_core,
        n_chunks=n_chunks,
        shared_tables=shared_tables,
    )


# --------------------------------------------------------------------------
# device program
# --------------------------------------------------------------------------
def _build(prep, weights, table_dt_name="float32"):
    from concourse import bacc, bass, mybir, tile
    from concourse.masks import make_identity

    f32 = mybir.dt.float32
    i32 = mybir.dt.int32
    tdt = getattr(mybir.dt, table_dt_name)
    Alu = mybir.AluOpType
    Act = mybir.ActivationFunctionType

    D = prep["D"]
    gpc = prep["gpc"]
    n_cores = prep["n_cores"]
    n_tiles = prep["n_tiles"]
    n_super = prep["n_super"]
    S_rows = prep["S_rows"]
    S_cols = prep["S_cols"]
    slots_u = prep["slots_u"]
    col_of = prep["col_of_super"]
    chunk_supers = prep["chunk_supers"]
    chunk_rows = prep["chunk_rows"]
    chunk_off = prep["chunk_off"]
    Z = prep["Z"]
    T_rows = prep["T_rows"]
    P_slots = prep["P_slots"]
    n_chunks = prep["n_chunks"]
    N = prep["N"]

    dma_eng = os.environ.get("GCN_DMA", "gpsimd")

    global DMA_ENGINE
    def DMA_ENGINE(nc_):
        return getattr(nc_, dma_eng)

    nc = bacc.Bacc(
        "TRN2",
        target_bir_lowering=False,
        debug=False,
        enable_asserts=False,
        num_devices=n_cores,
    )

    # ---- I/O ----
    xt_d = nc.dram_tensor("xt", [N + 1, D], tdt, kind="ExternalInput")
    idx1_d = nc.dram_tensor("idx1", [128, S_cols], i32, kind="ExternalInput")
    idx23_d = nc.dram_tensor("idx23", [128, S_cols], i32, kind="ExternalInput")
    pidx_d = nc.dram_tensor("pidx", [gpc, P_slots], i32, kind="ExternalInput")
    d2c_d = nc.dram_tensor("d2c", [128, n_tiles], f32, kind="ExternalInput")
    d1c_d = nc.dram_tensor("d1c", [128, n_tiles], f32, kind="ExternalInput")
    dvr_d = nc.dram_tensor("dvr", [1, S_rows], f32, kind="ExternalInput")
    gid_d = nc.dram_tensor("gid", [128, n_tiles], f32, kind="ExternalInput")
    icnt_d = nc.dram_tensor("icnt", [2, gpc], f32, kind="ExternalInput")
    w_d = [
        nc.dram_tensor(f"w{l}", [D, D], f32, kind="ExternalInput") for l in (1, 2, 3)
    ]
    br_d = [
        nc.dram_tensor(f"b{l}r", [1, D], f32, kind="ExternalInput") for l in (1, 2, 3)
    ]
    wcm_d = nc.dram_tensor("wcm", [D, 2], f32, kind="ExternalInput")
    wcx_d = nc.dram_tensor("wcx", [D, 2], f32, kind="ExternalInput")
    bc2_d = nc.dram_tensor("bc2", [2, 1], f32, kind="ExternalInput")
    out_d = nc.dram_tensor("out", [2, gpc], f32, kind="ExternalOutput")
    debug = int(os.environ.get("GCN_DEBUG", "0") or "0")
    if debug:
        dbg_h3 = nc.dram_tensor("dbg_h3", [S_rows + 1, D], f32, kind="ExternalOutput")
        dbg_sum = nc.dram_tensor("dbg_sum", [128, gpc], f32, kind="ExternalOutput")
        dbg_max = nc.dram_tensor("dbg_max", [gpc, 128], f32, kind="ExternalOutput")
    if debug >= 2:
        dbg_t2 = nc.dram_tensor("dbg_t2", [T_rows, D], tdt, kind="ExternalOutput")
        dbg_t3 = nc.dram_tensor("dbg_t3", [T_rows, D], tdt, kind="ExternalOutput")

    with tile.TileContext(nc) as tc:
        with (
            tc.tile_pool(name="constp", bufs=1) as constp,
            tc.tile_pool(name="gbp", bufs=3) as gbp,
            tc.tile_pool(name="accp", bufs=3) as accp,
            tc.tile_pool(name="miscp", bufs=3) as miscp,
            tc.tile_pool(name="idxp", bufs=2) as idxp,
            tc.tile_pool(name="psp", bufs=2, space="PSUM") as psp,
            tc.tile_pool(name="pst_p", bufs=1, space="PSUM") as pst_p,
            tc.tile_pool(name="dramp", bufs=1, space="DRAM") as dramp,
        ):
            # ---- constants ----
            ident = constp.tile([128, 128], f32, name="ident")
            make_identity(nc, ident[:])
            w_sb = []
            for l in range(3):
                wt = constp.tile([D, D], f32, name=f"w{l}sb")
                DMA_ENGINE(nc).dma_start(out=wt[:], in_=w_d[l].ap())
                w_sb.append(wt)
            br_sb = []
            for l in range(3):
                bt = constp.tile([1, D], f32, name=f"b{l}sb")
                DMA_ENGINE(nc).dma_start(out=bt[:], in_=br_d[l].ap())
                br_sb.append(bt)
            dvr = constp.tile([1, S_rows], f32, name="dvr_sb")
            DMA_ENGINE(nc).dma_start(out=dvr[:], in_=dvr_d.ap())
            d2c = constp.tile([128, n_tiles], f32, name="d2c_sb")
            DMA_ENGINE(nc).dma_start(out=d2c[:], in_=d2c_d.ap())
            d1c = constp.tile([128, n_tiles], f32, name="d1c_sb")
            DMA_ENGINE(nc).dma_start(out=d1c[:], in_=d1c_d.ap())
            gid = constp.tile([128, n_tiles], f32, name="gid_sb")
            DMA_ENGINE(nc).dma_start(out=gid[:], in_=gid_d.ap())
            icnt = constp.tile([2, gpc], f32, name="icnt_sb")
            DMA_ENGINE(nc).dma_start(out=icnt[:], in_=icnt_d.ap())
            wcm = constp.tile([D, 2], f32, name="wcm_sb")
            DMA_ENGINE(nc).dma_start(out=wcm[:], in_=wcm_d.ap())
            wcx = constp.tile([D, 2], f32, name="wcx_sb")
            DMA_ENGINE(nc).dma_start(out=wcx[:], in_=wcx_d.ap())
            bc2 = constp.tile([2, 1], f32, name="bc2_sb")
            DMA_ENGINE(nc).dma_start(out=bc2[:], in_=bc2_d.ap())
            pidx = constp.tile([gpc, P_slots], i32, name="pidx_sb")
            DMA_ENGINE(nc).dma_start(out=pidx[:], in_=pidx_d.ap())
            ones1 = constp.tile([1, 128], f32, name="ones1")
            nc.vector.memset(ones1[:], 1.0)
            iotag = constp.tile([128, gpc], f32, name="iotag")
            nc.gpsimd.iota(
                iotag[:],
                pattern=[[1, gpc]],
                channel_multiplier=0,
                allow_small_or_imprecise_dtypes=True,
            )
            zrow = constp.tile([1, D], tdt, name="zrow")
            nc.vector.memset(zrow[:], 0.0)
            nrow = constp.tile([1, D], f32, name="nrow")
            nc.vector.memset(nrow[:], -3.0e38)
            sumT = constp.tile([128, gpc], f32, name="sumT")
            nc.vector.memset(sumT[:], 0.0)

            # ---- DRAM scratch ----
            shared_tables = prep["shared_tables"]
            tbl_space = "Shared" if shared_tables else "Local"
            table = {
                2: dramp.tile([T_rows, D], tdt, name="table2", addr_space=tbl_space),
                3: dramp.tile([T_rows, D], tdt, name="table3", addr_space=tbl_space),
            }
            h3s = dramp.tile([S_rows + 1, D], f32, name="h3s")
            bounce = {
                l: [
                    dramp.tile([chunk_rows[c], D], tdt, name=f"bnc{l}_{c}")
                    for c in range(n_chunks)
                ]
                for l in (2, 3)
            }
            if not shared_tables:
                DMA_ENGINE(nc).dma_start(out=table[2][Z : Z + 1, :], in_=zrow[:])
                DMA_ENGINE(nc).dma_start(out=table[3][Z : Z + 1, :], in_=zrow[:])
            DMA_ENGINE(nc).dma_start(out=h3s[S_rows : S_rows + 1, :], in_=nrow[:])

            # ---- three GCN layers ----
            for layer in (1, 2, 3):
                src_ap = xt_d.ap() if layer == 1 else table[layer]
                idx_dram = idx1_d if layer == 1 else idx23_d
                dcol = d2c if layer < 3 else d1c
                w = w_sb[layer - 1]
                br = br_sb[layer - 1]

                for c in range(n_chunks):
                    cs = chunk_supers[c]
                    if len(cs) == 0:
                        continue
                    u0, u1 = int(cs[0]), int(cs[-1]) + 1
                    cc0, cc1 = int(col_of[u0]), int(col_of[u1])
                    idxt = idxp.tile([128, cc1 - cc0], i32, tag="idxt")
                    DMA_ENGINE(nc).dma_start(out=idxt[:], in_=idx_dram.ap()[:, cc0:cc1])

                    for u in range(u0, u1):
                        su = int(slots_u[u])
                        base = int(col_of[u]) - cc0
                        acc = accp.tile([128, 256], f32, tag="acc")
                        # HW indirect DMA only honors ONE index per partition
                        # per instruction (multi-index APs stream consecutive
                        # rows instead) — issue one gather per slot-half.
                        done = 0
                        while done < su:
                            g = min(GPAIR, su - done)
                            gb = gbp.tile([128, GPAIR * 256], tdt, tag="gb")
                            for j in range(g):
                                for h2 in (0, 1):
                                    nc.gpsimd.indirect_dma_start(
                                        out=gb[
                                            :, j * 256 + h2 * 128 : j * 256 + (h2 + 1) * 128
                                        ],
                                        out_offset=None,
                                        in_=src_ap,
                                        in_offset=bass.IndirectOffsetOnAxis(
                                            ap=idxt[
                                                :,
                                                base + 2 * (done + j) + h2
                                                : base + 2 * (done + j) + h2 + 1,
                                            ],
                                            axis=0,
                                        ),
                                    )
                            for j in range(g):
                                sl = gb[:, j * 256 : (j + 1) * 256]
                                if done + j == 0:
                                    nc.vector.tensor_copy(out=acc[:], in_=sl)
                                else:
                                    nc.vector.tensor_tensor(
                                        out=acc[:], in0=acc[:], in1=sl, op=Alu.add
                                    )
                            done += g

                        for h in (0, 1):
                            t = 2 * u + h
                            diag = miscp.tile([128, 128], f32, tag="diag")
                            nc.vector.tensor_scalar_mul(
                                out=diag[:], in0=ident[:], scalar1=dcol[:, t : t + 1]
                            )
                            ps1 = psp.tile([128, 128], f32, tag="ps1")
                            nc.tensor.matmul(
                                out=ps1[:],
                                lhsT=acc[:, h * 128 : (h + 1) * 128],
                                rhs=diag[:],
                                start=True,
                                stop=True,
                            )
                            sT = miscp.tile([128, 128], f32, tag="sT")
                            nc.vector.tensor_copy(out=sT[:], in_=ps1[:])
                            ps2 = psp.tile([128, 128], f32, tag="ps2")
                            if layer < 3:
                                nc.tensor.matmul(
                                    out=ps2[:],
                                    lhsT=dvr[:, t * 128 : (t + 1) * 128],
                                    rhs=br[:],
                                    start=True,
                                    stop=False,
                                )
                                nc.tensor.matmul(
                                    out=ps2[:], lhsT=sT[:], rhs=w[:],
                                    start=False, stop=True,
                                )
                                tout = miscp.tile([128, 128], tdt, tag="tout")
                                nc.scalar.activation(
                                    out=tout[:], in_=ps2[:], func=Act.Relu
                                )
                                r0 = (t - 2 * u0) * 128
                                DMA_ENGINE(nc).dma_start(
                                    out=bounce[layer + 1][c][r0 : r0 + 128, :],
                                    in_=tout[:],
                                )
                            else:
                                nc.tensor.matmul(
                                    out=ps2[:], lhsT=ones1[:], rhs=br[:],
                                    start=True, stop=False,
                                )
                                nc.tensor.matmul(
                                    out=ps2[:], lhsT=sT[:], rhs=w[:],
                                    start=False, stop=True,
                                )
                                h3t = miscp.tile([128, 128], f32, tag="tout")
                                nc.vector.tensor_copy(out=h3t[:], in_=ps2[:])
                                DMA_ENGINE(nc).dma_start(
                                    out=h3s[t * 128 : (t + 1) * 128, :], in_=h3t[:]
                                )
                                stile = miscp.tile([128, gpc], f32, tag="stile")
                                nc.vector.tensor_tensor(
                                    out=stile[:],
                                    in0=gid[:, t : t + 1].to_broadcast([128, gpc]),
                                    in1=iotag[:],
                                    op=Alu.is_equal,
                                )
                                pst = pst_p.tile([128, gpc], f32, tag="pst")
                                nc.tensor.matmul(
                                    out=pst[:], lhsT=h3t[:], rhs=stile[:],
                                    start=True, stop=True,
                                )
                                nc.vector.tensor_tensor(
                                    out=sumT[:], in0=sumT[:], in1=pst[:], op=Alu.add
                                )

                    if layer < 3:
                        nc.gpsimd.collective_compute(
                            "AllGather",
                            Alu.bypass,
                            replica_groups=[list(range(n_cores))],
                            ins=[bounce[layer + 1][c][:].opt()],
                            outs=[
                                table[layer + 1][
                                    int(chunk_off[c]) : int(chunk_off[c])
                                    + n_cores * chunk_rows[c],
                                    :,
                                ].opt()
                            ],
                        )

            # ---- max pooling over local h3 ----
            maxacc = constp.tile([gpc, 128], f32, name="maxacc")
            done = 0
            while done < P_slots:
                g = min(POOL_G, P_slots - done)
                pgb = gbp.tile([gpc, POOL_G * 128], f32, tag="pgb")
                for j in range(g):
                    nc.gpsimd.indirect_dma_start(
                        out=pgb[:, j * 128 : (j + 1) * 128],
                        out_offset=None,
                        in_=h3s,
                        in_offset=bass.IndirectOffsetOnAxis(
                            ap=pidx[:, done + j : done + j + 1], axis=0
                        ),
                    )
                for j in range(g):
                    sl = pgb[:, j * 128 : (j + 1) * 128]
                    if done + j == 0:
                        nc.vector.tensor_copy(out=maxacc[:], in_=sl)
                    else:
                        nc.vector.tensor_tensor(
                            out=maxacc[:], in0=maxacc[:], in1=sl, op=Alu.max
                        )
                done += g
            psmT = pst_p.tile([128, gpc], f32, tag="psmT")
            nc.tensor.transpose(
                out=psmT[:], in_=maxacc[:], identity=ident[:gpc, :gpc]
            )
            maxT = miscp.tile([128, gpc], f32, tag="maxT")
            nc.vector.tensor_copy(out=maxT[:], in_=psmT[:])

            # ---- classifier ----
            psz1 = pst_p.tile([2, gpc], f32, tag="psz1")
            nc.tensor.matmul(out=psz1[:], lhsT=wcm[:], rhs=sumT[:], start=True, stop=True)
            psz2 = pst_p.tile([2, gpc], f32, tag="psz2")
            nc.tensor.matmul(out=psz2[:], lhsT=wcx[:], rhs=maxT[:], start=True, stop=True)
            zt = miscp.tile([2, gpc], f32, tag="zt")
            nc.vector.tensor_tensor(out=zt[:], in0=psz1[:], in1=icnt[:], op=Alu.mult)
            nc.vector.tensor_tensor(out=zt[:], in0=zt[:], in1=psz2[:], op=Alu.add)
            nc.vector.tensor_scalar_add(out=zt[:], in0=zt[:], scalar1=bc2[:, :1])
            DMA_ENGINE(nc).dma_start(out=out_d.ap(), in_=zt[:])

            if debug:
                # bounce whole tables / scratch through SBUF tiles to outputs
                def dump(dst_ap, src_ap, rows, width, dt_):
                    for r0 in range(0, rows, 128):
                        r1 = min(r0 + 128, rows)
                        buf = miscp.tile([128, width], dt_, tag="dbgbuf")
                        DMA_ENGINE(nc).dma_start(
                            out=buf[: r1 - r0, :], in_=src_ap[r0:r1, :]
                        )
                        DMA_ENGINE(nc).dma_start(
                            out=dst_ap[r0:r1, :], in_=buf[: r1 - r0, :]
                        )

                if debug >= 2:
                    dump(dbg_t2.ap(), table[2], T_rows, D, tdt)
                    dump(dbg_t3.ap(), table[3], T_rows, D, tdt)
                dump(dbg_h3.ap(), h3s, S_rows + 1, D, f32)
                dbuf = miscp.tile([128, gpc], f32, tag="dbgs")
                nc.vector.tensor_copy(out=dbuf[:], in_=sumT[:])
                DMA_ENGINE(nc).dma_start(out=dbg_sum.ap(), in_=dbuf[:])
                dbuf2 = miscp.tile([gpc, 128], f32, tag="dbgm")
                nc.vector.tensor_copy(out=dbuf2[:], in_=maxacc[:])
                DMA_ENGINE(nc).dma_start(out=dbg_max.ap(), in_=dbuf2[:])

    return nc


def _in_maps(prep, weights, table_dt_name):
    np_tdt = np.float32 if table_dt_name == "float32" else None
    xt = prep["xt"]
    if table_dt_name == "bfloat16":
        import ml_dtypes

        np_tdt = ml_dtypes.bfloat16
    xt = xt.astype(np_tdt)
    W1, b1, W2, b2, W3, b3, Wc, bc = weights
    maps = []
    for pc in prep["per_core"]:
        maps.append(
            {
                "xt": xt,
                "idx1": pc["idx1"],
                "idx23": pc["idx23"],
                "pidx": pc["pidx"],
                "d2c": pc["d2c"],
                "d1c": pc["d1c"],
                "dvr": pc["dvr"],
                "gid": pc["gid"],
                "icnt": pc["icnt"],
                "w1": np.asarray(W1, np.float32),
                "w2": np.asarray(W2, np.float32),
                "w3": np.asarray(W3, np.float32),
                "b1r": np.asarray(b1, np.float32).reshape(1, -1),
                "b2r": np.asarray(b2, np.float32).reshape(1, -1),
                "b3r": np.asarray(b3, np.float32).reshape(1, -1),
                "wcm": np.asarray(Wc, np.float32)[: prep["D"]],
                "wcx": np.asarray(Wc, np.float32)[prep["D"] :],
                "bc2": np.asarray(bc, np.float32).reshape(2, 1),
                "icnt": pc["icnt"],
            }
        )
    return maps


# --------------------------------------------------------------------------
# entry point
# --------------------------------------------------------------------------
def kernel(x, edge_index, batch, W1, b1, W2, b2, W3, b3, Wc, bc):
    global LAST_RESULTS
    from concourse import bass_utils

    n_graphs = 512
    shared = os.environ.get("GCN_SHARED", "0") == "1"
    prep = _host_prep(
        x, edge_index, batch, N_CORES, n_graphs, N_CHUNKS, shared_tables=shared
    )
    weights = (W1, b1, W2, b2, W3, b3, Wc, bc)
    nc = _build(prep, weights, TABLE_DT)
    nc.compile()
    maps = _in_maps(prep, weights, TABLE_DT)
    res = bass_utils.run_bass_kernel_spmd(
        nc,
        maps,
        core_ids=list(range(N_CORES)),
        trace=os.environ.get("GCN_TRACE") == "1",
    )
    LAST_RESULTS = res
    outs = [res.results[c]["out"] for c in range(N_CORES)]
    return np.concatenate([np.asarray(o, np.float32).T for o in outs], 0)



# revision 9
# speedup vs baseline: 1.0021x; 1.0021x over previous
"""3-layer GCN + global mean/max pool + linear classifier on 8 Trainium2 NeuronCores.

Strategy (node-parallel, NOT edge-parallel):
  * Aggregate-first algebra: Agg(x @ W) == Agg(x) @ W, with the symmetric
    normalization folded into pre-scaled features  xt = dinv * x  so the
    per-edge multiply disappears:
        out[v] = dinv[v] * ( sum_{e: dst=v} xt[src_e] + xt[v] ) @ W + b
  * Shard dst nodes across 8 cores at graph-aligned boundaries (64 graphs /
    core).  Each core aggregates only its own ~12.5K dst rows (~400K edges),
    gathering source rows from a replicated feature table via indirect DMA
    (128 rows x 512B per "slot", batched ~1MB per DMA instruction).
  * Per-core dst nodes are sorted by in-degree so fixed-slot tiles have
    almost no padding.  All per-core variation (indices, scales, graph ids)
    is input data => one SPMD program for all cores.
  * After layers 1-2 each core's output shard is AllGather'd (in chunks, to
    overlap with compute) into the next layer's gather table.  Layer 3 output
    stays local: pooling only needs the core's own 64 graphs.
  * Sum-pool via PE matmul with a data-driven graph-indicator matrix; max-pool
    via a second small indirect gather pass over the local h3 scratch.

kernel(**inputs) takes the full unsharded inputs and returns the full
[512, 2] output.
"""

import os
import sys

import numpy as np

sys.path.insert(0, "/opt/trn_rl_repo")

N_CORES = 8
GPAIR = 8  # pair-slots per main gather DMA (gather buf = [128, GPAIR*256])
POOL_G = 16  # slots per pooling gather DMA
N_CHUNKS = 4  # allgather chunks per layer
TABLE_DT = os.environ.get("GCN_TABLE_DT", "float32")  # float32 | bfloat16

LAST_RESULTS = None  # BassKernelResults of the most recent run (for test.py)


# --------------------------------------------------------------------------
# host-side graph preprocessing
# --------------------------------------------------------------------------
def _host_prep(x, edge_index, batch, n_cores, n_graphs, n_chunks, shared_tables=False):
    """Compute all per-core index/scale arrays and the layout metadata.

    shared_tables: layer-2/3 gather tables are Shared-address-space DRAM
    written by a single mesh AllGather (needs n_cores > 4).  The zero row is
    then a pad row inside some core's stripe (pad rows compute to exact 0).
    """
    if shared_tables:
        n_chunks = 1
    x = np.asarray(x, dtype=np.float32)
    edge_index = np.asarray(edge_index)
    batch = np.asarray(batch).astype(np.int64)
    N, D = x.shape
    E = edge_index.shape[1]
    gpc = n_graphs // n_cores

    src = edge_index[0].astype(np.int64)
    dst = edge_index[1].astype(np.int64)
    indeg = np.bincount(dst, minlength=N).astype(np.int64)
    deg = (indeg + 1).astype(np.float64)  # + self loop
    dinv = (1.0 / np.sqrt(deg)).astype(np.float32)

    xt = x * dinv[:, None]
    xt = np.concatenate([xt, np.zeros((1, D), np.float32)], 0)  # zero row at N
    # self rows are streamed via contiguous DMA (not gathered): per-core
    # perm-ordered copy of xt, padded to S_rows

    # graph boundaries (batch is sorted)
    gstart = np.searchsorted(batch, np.arange(n_graphs + 1)).astype(np.int64)
    B = gstart[np.arange(n_cores + 1) * gpc]  # core node boundaries

    # per-core degree-sorted permutation of owned nodes
    perms = []
    for i in range(n_cores):
        nodes = np.arange(B[i], B[i + 1])
        perms.append(nodes[np.argsort(-indeg[nodes], kind="stable")])
    n_own = np.array([len(p) for p in perms])
    n_tiles = int(np.ceil((n_own.max() + (1 if shared_tables else 0)) / 128))
    n_tiles += n_tiles % 2  # even, for pair supertiles
    S_rows = n_tiles * 128
    n_super = n_tiles // 2

    # localpos[v] = position of node v within its core's permuted layout
    localpos = np.zeros(N, dtype=np.int64)
    for p in perms:
        localpos[p] = np.arange(len(p))

    # slots per supertile: max in-degree among its 256 nodes, maxed across
    # cores (static SPMD program).  Self rows are NOT gathered (streamed
    # contiguously instead), so no +1.
    slots_u = np.zeros(n_super, dtype=np.int64)
    for i in range(n_cores):
        dg = np.zeros(S_rows, dtype=np.int64)
        dg[: n_own[i]] = indeg[perms[i]]
        m = dg.reshape(n_super, 256).max(1)
        slots_u = np.maximum(slots_u, m)
    M = int(slots_u.max())
    col_of_super = np.concatenate([[0], np.cumsum(2 * slots_u)]).astype(np.int64)
    S_cols = int(col_of_super[-1])

    # edge lists sorted by dst, with per-dst rank
    eo = np.argsort(dst, kind="stable")
    ds, ss = dst[eo], src[eo]
    eptr = np.searchsorted(ds, np.arange(N + 1))
    erank = np.arange(E, dtype=np.int64) - eptr[ds]

    # chunk layout over supertiles
    chunk_supers = np.array_split(np.arange(n_super), n_chunks)
    chunk_u0 = [int(cs[0]) if len(cs) else 0 for cs in chunk_supers]
    chunk_rows = [len(cs) * 256 for cs in chunk_supers]
    chunk_off = np.concatenate([[0], np.cumsum(np.array(chunk_rows) * n_cores)])
    chunk_of_super = np.zeros(n_super, dtype=np.int64)
    for c, cs in enumerate(chunk_supers):
        chunk_of_super[cs] = c

    # table position of each node for layers 2/3
    chunk_rows_arr = np.array(chunk_rows, dtype=np.int64)
    chunk_base = np.array([chunk_u0[cc] * 256 for cc in range(n_chunks)], dtype=np.int64)

    def pos_of(core, r):
        c = chunk_of_super[np.asarray(r) // 256]
        return chunk_off[c] + core * chunk_rows_arr[c] + (np.asarray(r) - chunk_base[c])

    if shared_tables:
        # zero row = first pad row of the core with the most padding
        k0 = int(np.argmin(n_own))
        assert n_own[k0] < S_rows, "no pad rows available for the zero row"
        Z = int(pos_of(k0, n_own[k0]))
        T_rows = int(chunk_off[-1])
    else:
        Z = int(chunk_off[-1])  # extra zero row appended past all stripes
        T_rows = Z + 1

    pos23 = np.full(N + 1, Z, dtype=np.int64)
    for i in range(n_cores):
        r = np.arange(n_own[i])
        pos23[perms[i]] = pos_of(i, r)

    per_core = []
    for i in range(n_cores):
        # node-id grid [S_rows, M]; -1 = padding (self rows streamed, not
        # gathered)
        grid = np.full((S_rows, max(M, 1)), -1, dtype=np.int64)
        m = (ds >= B[i]) & (ds < B[i + 1])
        grid[localpos[ds[m]], erank[m]] = ss[m]

        def grid_to_idx(posmap, padpos):
            cols = []
            for u in range(n_super):
                su = int(slots_u[u])
                sub = grid[u * 256 : (u + 1) * 256, :su]
                p = np.where(sub < 0, padpos, posmap[np.clip(sub, 0, None)])
                cols.append(
                    p.reshape(2, 128, su).transpose(1, 2, 0).reshape(128, 2 * su)
                )
            return np.ascontiguousarray(
                np.concatenate(cols, axis=1).astype(np.int32)
            )

        idx1 = grid_to_idx(np.arange(N + 1, dtype=np.int64), N)
        idx23 = grid_to_idx(pos23, Z)

        # per-tile scale columns (perm order, pads = 0)
        dv = np.zeros(S_rows, dtype=np.float32)
        dv[: n_own[i]] = dinv[perms[i]]
        d2c = np.ascontiguousarray((dv * dv).reshape(n_tiles, 128).T)
        d1c = np.ascontiguousarray(dv.reshape(n_tiles, 128).T)
        dvr = dv.reshape(1, S_rows)

        # graph id (local) of each perm row; pads = -1
        gl = np.full(S_rows, -1.0, dtype=np.float32)
        gl[: n_own[i]] = (batch[perms[i]] - i * gpc).astype(np.float32)
        gid = np.ascontiguousarray(gl.reshape(n_tiles, 128).T)

        # per-core perm-ordered self rows (streamed contiguously at layer 1)
        xts = np.zeros((S_rows, D), np.float32)
        xts[: n_own[i]] = xt[perms[i]]

        per_core.append(
            dict(idx1=idx1, idx23=idx23, d2c=d2c, d1c=d1c, dvr=dvr, gid=gid,
                 xts=xts)
        )

    # pooling: P_slots = max graph size (global); pool idx [gpc, P_slots]
    cnt_all = np.diff(gstart)
    P_slots = int(cnt_all.max())
    P_slots = -(-P_slots // POOL_G) * POOL_G  # round up to POOL_G
    for i in range(n_cores):
        pidx = np.full((gpc, P_slots), S_rows, dtype=np.int32)  # -inf row
        cnt = np.zeros(gpc, dtype=np.int64)
        for g in range(gpc):
            s, e = gstart[i * gpc + g], gstart[i * gpc + g + 1]
            cnt[g] = e - s
            pidx[g, : e - s] = localpos[np.arange(s, e)]
        per_core[i]["pidx"] = pidx
        ic = np.where(cnt > 0, 1.0 / np.maximum(cnt, 1), 0.0).astype(np.float32)
        per_core[i]["icnt"] = np.ascontiguousarray(
            np.broadcast_to(ic[None, :], (2, gpc))
        )

    return dict(
        N=N,
        D=D,
        gpc=gpc,
        n_cores=n_cores,
        n_tiles=n_tiles,
        n_super=n_super,
        S_rows=S_rows,
        S_cols=S_cols,
        slots_u=slots_u,
        col_of_super=col_of_super,
        chunk_supers=chunk_supers,
        chunk_u0=chunk_u0,
        chunk_rows=chunk_rows,
        chunk_off=chunk_off,
        Z=Z,
        T_rows=T_rows,
        P_slots=P_slots,
        xt=xt,
        per_core=per_core,
        n_chunks=n_chunks,
        shared_tables=shared_tables,
    )


# --------------------------------------------------------------------------
# device program
# --------------------------------------------------------------------------
def _build(prep, weights, table_dt_name="float32"):
    from concourse import bacc, bass, mybir, tile
    from concourse.masks import make_identity

    f32 = mybir.dt.float32
    i32 = mybir.dt.int32
    tdt = getattr(mybir.dt, table_dt_name)
    Alu = mybir.AluOpType
    Act = mybir.ActivationFunctionType

    D = prep["D"]
    gpc = prep["gpc"]
    n_cores = prep["n_cores"]
    n_tiles = prep["n_tiles"]
    n_super = prep["n_super"]
    S_rows = prep["S_rows"]
    S_cols = prep["S_cols"]
    slots_u = prep["slots_u"]
    col_of = prep["col_of_super"]
    chunk_supers = prep["chunk_supers"]
    chunk_rows = prep["chunk_rows"]
    chunk_off = prep["chunk_off"]
    Z = prep["Z"]
    T_rows = prep["T_rows"]
    P_slots = prep["P_slots"]
    n_chunks = prep["n_chunks"]
    N = prep["N"]

    dma_eng = os.environ.get("GCN_DMA", "gpsimd")

    global DMA_ENGINE
    def DMA_ENGINE(nc_):
        return getattr(nc_, dma_eng)

    nc = bacc.Bacc(
        "TRN2",
        target_bir_lowering=False,
        debug=False,
        enable_asserts=False,
        num_devices=n_cores,
    )

    # ---- I/O ----
    xt_d = nc.dram_tensor("xt", [N + 1, D], tdt, kind="ExternalInput")
    xts_d = nc.dram_tensor("xts", [S_rows, D], tdt, kind="ExternalInput")
    iot_d = nc.dram_tensor("iot", [128, gpc], f32, kind="ExternalInput")
    idx1_d = nc.dram_tensor("idx1", [128, S_cols], i32, kind="ExternalInput")
    idx23_d = nc.dram_tensor("idx23", [128, S_cols], i32, kind="ExternalInput")
    pidx_d = nc.dram_tensor("pidx", [gpc, P_slots], i32, kind="ExternalInput")
    d2c_d = nc.dram_tensor("d2c", [128, n_tiles], f32, kind="ExternalInput")
    d1c_d = nc.dram_tensor("d1c", [128, n_tiles], f32, kind="ExternalInput")
    dvr_d = nc.dram_tensor("dvr", [1, S_rows], f32, kind="ExternalInput")
    gid_d = nc.dram_tensor("gid", [128, n_tiles], f32, kind="ExternalInput")
    icnt_d = nc.dram_tensor("icnt", [2, gpc], f32, kind="ExternalInput")
    w_d = [
        nc.dram_tensor(f"w{l}", [D, D], f32, kind="ExternalInput") for l in (1, 2, 3)
    ]
    br_d = [
        nc.dram_tensor(f"b{l}r", [1, D], f32, kind="ExternalInput") for l in (1, 2, 3)
    ]
    wcm_d = nc.dram_tensor("wcm", [D, 2], f32, kind="ExternalInput")
    wcx_d = nc.dram_tensor("wcx", [D, 2], f32, kind="ExternalInput")
    bc2_d = nc.dram_tensor("bc2", [2, 1], f32, kind="ExternalInput")
    out_d = nc.dram_tensor("out", [2, gpc], f32, kind="ExternalOutput")
    debug = int(os.environ.get("GCN_DEBUG", "0") or "0")
    if debug:
        dbg_h3 = nc.dram_tensor("dbg_h3", [S_rows + 1, D], f32, kind="ExternalOutput")
        dbg_sum = nc.dram_tensor("dbg_sum", [128, gpc], f32, kind="ExternalOutput")
        dbg_max = nc.dram_tensor("dbg_max", [gpc, 128], f32, kind="ExternalOutput")
    if debug >= 2:
        dbg_t2 = nc.dram_tensor("dbg_t2", [T_rows, D], tdt, kind="ExternalOutput")
        dbg_t3 = nc.dram_tensor("dbg_t3", [T_rows, D], tdt, kind="ExternalOutput")

    with tile.TileContext(nc) as tc:
        with (
            tc.tile_pool(name="constp", bufs=1) as constp,
            tc.tile_pool(name="gbp", bufs=5) as gbp,
            tc.tile_pool(name="accp", bufs=3) as accp,
            tc.tile_pool(name="miscp", bufs=4) as miscp,
            tc.tile_pool(name="selfp", bufs=4) as selfp,
            tc.tile_pool(name="idxp", bufs=2) as idxp,
            tc.tile_pool(name="psp", bufs=3, space="PSUM") as psp,
            tc.tile_pool(name="pst_p", bufs=1, space="PSUM") as pst_p,
            tc.tile_pool(name="dramp", bufs=1, space="DRAM") as dramp,
        ):
            # ---- constants ----
            ident = constp.tile([128, 128], f32, name="ident")
            make_identity(nc, ident[:])
            w_sb = []
            for l in range(3):
                wt = constp.tile([D, D], f32, name=f"w{l}sb")
                DMA_ENGINE(nc).dma_start(out=wt[:], in_=w_d[l].ap())
                w_sb.append(wt)
            br_sb = []
            for l in range(3):
                bt = constp.tile([1, D], f32, name=f"b{l}sb")
                DMA_ENGINE(nc).dma_start(out=bt[:], in_=br_d[l].ap())
                br_sb.append(bt)
            dvr = constp.tile([1, S_rows], f32, name="dvr_sb")
            DMA_ENGINE(nc).dma_start(out=dvr[:], in_=dvr_d.ap())
            d2c = constp.tile([128, n_tiles], f32, name="d2c_sb")
            DMA_ENGINE(nc).dma_start(out=d2c[:], in_=d2c_d.ap())
            d1c = constp.tile([128, n_tiles], f32, name="d1c_sb")
            DMA_ENGINE(nc).dma_start(out=d1c[:], in_=d1c_d.ap())
            gid = constp.tile([128, n_tiles], f32, name="gid_sb")
            DMA_ENGINE(nc).dma_start(out=gid[:], in_=gid_d.ap())
            icnt = constp.tile([2, gpc], f32, name="icnt_sb")
            DMA_ENGINE(nc).dma_start(out=icnt[:], in_=icnt_d.ap())
            wcm = constp.tile([D, 2], f32, name="wcm_sb")
            DMA_ENGINE(nc).dma_start(out=wcm[:], in_=wcm_d.ap())
            wcx = constp.tile([D, 2], f32, name="wcx_sb")
            DMA_ENGINE(nc).dma_start(out=wcx[:], in_=wcx_d.ap())
            bc2 = constp.tile([2, 1], f32, name="bc2_sb")
            DMA_ENGINE(nc).dma_start(out=bc2[:], in_=bc2_d.ap())
            pidx = constp.tile([gpc, P_slots], i32, name="pidx_sb")
            DMA_ENGINE(nc).dma_start(out=pidx[:], in_=pidx_d.ap())
            ones1 = constp.tile([1, 128], f32, name="ones1")
            nc.vector.memset(ones1[:], 1.0)
            iotag = constp.tile([128, gpc], f32, name="iotag")
            DMA_ENGINE(nc).dma_start(out=iotag[:], in_=iot_d.ap())
            zrow = constp.tile([1, D], tdt, name="zrow")
            nc.vector.memset(zrow[:], 0.0)
            nrow = constp.tile([1, D], f32, name="nrow")
            nc.vector.memset(nrow[:], -3.0e38)
            sumT = constp.tile([128, gpc], f32, name="sumT")
            nc.vector.memset(sumT[:], 0.0)

            # ---- DRAM scratch ----
            shared_tables = prep["shared_tables"]
            tbl_space = "Shared" if shared_tables else "Local"
            table = {
                2: dramp.tile([T_rows, D], tdt, name="table2", addr_space=tbl_space),
                3: dramp.tile([T_rows, D], tdt, name="table3", addr_space=tbl_space),
            }
            h3s = dramp.tile([S_rows + 1, D], f32, name="h3s")
            bounce = {
                l: [
                    dramp.tile([chunk_rows[c], D], tdt, name=f"bnc{l}_{c}")
                    for c in range(n_chunks)
                ]
                for l in (2, 3)
            }
            if not shared_tables:
                DMA_ENGINE(nc).dma_start(out=table[2][Z : Z + 1, :], in_=zrow[:])
                DMA_ENGINE(nc).dma_start(out=table[3][Z : Z + 1, :], in_=zrow[:])
            DMA_ENGINE(nc).dma_start(out=h3s[S_rows : S_rows + 1, :], in_=nrow[:])

            # ---- three GCN layers ----
            for layer in (1, 2, 3):
                src_ap = xt_d.ap() if layer == 1 else table[layer]
                idx_dram = idx1_d if layer == 1 else idx23_d
                dcol = d2c if layer < 3 else d1c
                w = w_sb[layer - 1]
                br = br_sb[layer - 1]

                for c in range(n_chunks):
                    cs = chunk_supers[c]
                    if len(cs) == 0:
                        continue
                    u0, u1 = int(cs[0]), int(cs[-1]) + 1
                    cc0, cc1 = int(col_of[u0]), int(col_of[u1])
                    if cc1 > cc0:
                        idxt = idxp.tile([128, cc1 - cc0], i32, tag="idxt")
                        DMA_ENGINE(nc).dma_start(
                            out=idxt[:], in_=idx_dram.ap()[:, cc0:cc1]
                        )

                    for u in range(u0, u1):
                        su = int(slots_u[u])
                        base = int(col_of[u]) - cc0
                        acc = accp.tile([128, 256], f32, tag="acc")
                        # init acc from the contiguous self-row stream: layer 1
                        # reads perm-ordered xt, layers 2/3 re-read this core's
                        # own bounce rows from the previous layer.
                        selfb = selfp.tile([128, 256], tdt, tag="selfb")
                        for h in (0, 1):
                            t = 2 * u + h
                            if layer == 1:
                                self_src = xts_d.ap()[t * 128 : (t + 1) * 128, :]
                            else:
                                r0 = (t - 2 * u0) * 128
                                self_src = bounce[layer][c][r0 : r0 + 128, :]
                            DMA_ENGINE(nc).dma_start(
                                out=selfb[:, h * 128 : (h + 1) * 128], in_=self_src
                            )
                        nc.vector.tensor_copy(out=acc[:], in_=selfb[:])
                        # HW indirect DMA only honors ONE index per partition
                        # per instruction (multi-index APs stream consecutive
                        # rows instead) — issue one gather per slot-half.
                        done = 0
                        while done < su:
                            g = min(GPAIR, su - done)
                            gb = gbp.tile([128, GPAIR * 256], tdt, tag="gb")
                            for j in range(g):
                                for h2 in (0, 1):
                                    nc.gpsimd.indirect_dma_start(
                                        out=gb[
                                            :, j * 256 + h2 * 128 : j * 256 + (h2 + 1) * 128
                                        ],
                                        out_offset=None,
                                        in_=src_ap,
                                        in_offset=bass.IndirectOffsetOnAxis(
                                            ap=idxt[
                                                :,
                                                base + 2 * (done + j) + h2
                                                : base + 2 * (done + j) + h2 + 1,
                                            ],
                                            axis=0,
                                        ),
                                    )
                            for j in range(g):
                                sl = gb[:, j * 256 : (j + 1) * 256]
                                nc.vector.tensor_tensor(
                                    out=acc[:], in0=acc[:], in1=sl, op=Alu.add
                                )
                            done += g

                        for h in (0, 1):
                            t = 2 * u + h
                            diag = miscp.tile([128, 128], f32, tag="diag")
                            nc.vector.tensor_scalar_mul(
                                out=diag[:], in0=ident[:], scalar1=dcol[:, t : t + 1]
                            )
                            ps1 = psp.tile([128, 128], f32, tag="ps1")
                            nc.tensor.matmul(
                                out=ps1[:],
                                lhsT=acc[:, h * 128 : (h + 1) * 128],
                                rhs=diag[:],
                                start=True,
                                stop=True,
                            )
                            sT = miscp.tile([128, 128], f32, tag="sT")
                            nc.vector.tensor_copy(out=sT[:], in_=ps1[:])
                            ps2 = psp.tile([128, 128], f32, tag="ps2")
                            if layer < 3:
                                nc.tensor.matmul(
                                    out=ps2[:],
                                    lhsT=dvr[:, t * 128 : (t + 1) * 128],
                                    rhs=br[:],
                                    start=True,
                                    stop=False,
                                )
                                nc.tensor.matmul(
                                    out=ps2[:], lhsT=sT[:], rhs=w[:],
                                    start=False, stop=True,
                                )
                                tout = miscp.tile([128, 128], tdt, tag="tout")
                                nc.scalar.activation(
                                    out=tout[:], in_=ps2[:], func=Act.Relu
                                )
                                r0 = (t - 2 * u0) * 128
                                DMA_ENGINE(nc).dma_start(
                                    out=bounce[layer + 1][c][r0 : r0 + 128, :],
                                    in_=tout[:],
                                )
                            else:
                                nc.tensor.matmul(
                                    out=ps2[:], lhsT=ones1[:], rhs=br[:],
                                    start=True, stop=False,
                                )
                                nc.tensor.matmul(
                                    out=ps2[:], lhsT=sT[:], rhs=w[:],
                                    start=False, stop=True,
                                )
                                h3t = miscp.tile([128, 128], f32, tag="tout")
                                nc.vector.tensor_copy(out=h3t[:], in_=ps2[:])
                                DMA_ENGINE(nc).dma_start(
                                    out=h3s[t * 128 : (t + 1) * 128, :], in_=h3t[:]
                                )
                                stile = miscp.tile([128, gpc], f32, tag="stile")
                                nc.vector.tensor_tensor(
                                    out=stile[:],
                                    in0=gid[:, t : t + 1].to_broadcast([128, gpc]),
                                    in1=iotag[:],
                                    op=Alu.is_equal,
                                )
                                pst = pst_p.tile([128, gpc], f32, tag="pst")
                                nc.tensor.matmul(
                                    out=pst[:], lhsT=h3t[:], rhs=stile[:],
                                    start=True, stop=True,
                                )
                                nc.vector.tensor_tensor(
                                    out=sumT[:], in0=sumT[:], in1=pst[:], op=Alu.add
                                )

                    if layer < 3:
                        nc.gpsimd.collective_compute(
                            "AllGather",
                            Alu.bypass,
                            replica_groups=[list(range(n_cores))],
                            ins=[bounce[layer + 1][c][:].opt()],
                            outs=[
                                table[layer + 1][
                                    int(chunk_off[c]) : int(chunk_off[c])
                                    + n_cores * chunk_rows[c],
                                    :,
                                ].opt()
                            ],
                        )

            # ---- max pooling over local h3 ----
            maxacc = constp.tile([gpc, 128], f32, name="maxacc")
            done = 0
            while done < P_slots:
                g = min(POOL_G, P_slots - done)
                pgb = gbp.tile([gpc, POOL_G * 128], f32, tag="pgb")
                for j in range(g):
                    nc.gpsimd.indirect_dma_start(
                        out=pgb[:, j * 128 : (j + 1) * 128],
                        out_offset=None,
                        in_=h3s,
                        in_offset=bass.IndirectOffsetOnAxis(
                            ap=pidx[:, done + j : done + j + 1], axis=0
                        ),
                    )
                for j in range(g):
                    sl = pgb[:, j * 128 : (j + 1) * 128]
                    if done + j == 0:
                        nc.vector.tensor_copy(out=maxacc[:], in_=sl)
                    else:
                        nc.vector.tensor_tensor(
                            out=maxacc[:], in0=maxacc[:], in1=sl, op=Alu.max
                        )
                done += g
            psmT = pst_p.tile([128, gpc], f32, tag="psmT")
            nc.tensor.transpose(
                out=psmT[:], in_=maxacc[:], identity=ident[:gpc, :gpc]
            )
            maxT = miscp.tile([128, gpc], f32, tag="maxT")
            nc.vector.tensor_copy(out=maxT[:], in_=psmT[:])

            # ---- classifier ----
            psz1 = pst_p.tile([2, gpc], f32, tag="psz1")
            nc.tensor.matmul(out=psz1[:], lhsT=wcm[:], rhs=sumT[:], start=True, stop=True)
            psz2 = pst_p.tile([2, gpc], f32, tag="psz2")
            nc.tensor.matmul(out=psz2[:], lhsT=wcx[:], rhs=maxT[:], start=True, stop=True)
            zt = miscp.tile([2, gpc], f32, tag="zt")
            nc.vector.tensor_tensor(out=zt[:], in0=psz1[:], in1=icnt[:], op=Alu.mult)
            nc.vector.tensor_tensor(out=zt[:], in0=zt[:], in1=psz2[:], op=Alu.add)
            nc.vector.tensor_scalar_add(out=zt[:], in0=zt[:], scalar1=bc2[:, :1])
            DMA_ENGINE(nc).dma_start(out=out_d.ap(), in_=zt[:])

            if debug:
                # bounce whole tables / scratch through SBUF tiles to outputs
                def dump(dst_ap, src_ap, rows, width, dt_):
                    for r0 in range(0, rows, 128):
                        r1 = min(r0 + 128, rows)
                        buf = miscp.tile([128, width], dt_, tag="dbgbuf")
                        DMA_ENGINE(nc).dma_start(
                            out=buf[: r1 - r0, :], in_=src_ap[r0:r1, :]
                        )
                        DMA_ENGINE(nc).dma_start(
                            out=dst_ap[r0:r1, :], in_=buf[: r1 - r0, :]
                        )

                if debug >= 2:
                    dump(dbg_t2.ap(), table[2], T_rows, D, tdt)
                    dump(dbg_t3.ap(), table[3], T_rows, D, tdt)
                dump(dbg_h3.ap(), h3s, S_rows + 1, D, f32)
                dbuf = miscp.tile([128, gpc], f32, tag="dbgs")
                nc.vector.tensor_copy(out=dbuf[:], in_=sumT[:])
                DMA_ENGINE(nc).dma_start(out=dbg_sum.ap(), in_=dbuf[:])
                dbuf2 = miscp.tile([gpc, 128], f32, tag="dbgm")
                nc.vector.tensor_copy(out=dbuf2[:], in_=maxacc[:])
                DMA_ENGINE(nc).dma_start(out=dbg_max.ap(), in_=dbuf2[:])

    return nc


def _in_maps(prep, weights, table_dt_name):
    np_tdt = np.float32 if table_dt_name == "float32" else None
    xt = prep["xt"]
    if table_dt_name == "bfloat16":
        import ml_dtypes

        np_tdt = ml_dtypes.bfloat16
    xt = xt.astype(np_tdt)
    W1, b1, W2, b2, W3, b3, Wc, bc = weights
    maps = []
    for pc in prep["per_core"]:
        maps.append(
            {
                "xt": xt,
                "idx1": pc["idx1"],
                "idx23": pc["idx23"],
                "pidx": pc["pidx"],
                "d2c": pc["d2c"],
                "d1c": pc["d1c"],
                "dvr": pc["dvr"],
                "gid": pc["gid"],
                "icnt": pc["icnt"],
                "w1": np.asarray(W1, np.float32),
                "w2": np.asarray(W2, np.float32),
                "w3": np.asarray(W3, np.float32),
                "b1r": np.asarray(b1, np.float32).reshape(1, -1),
                "b2r": np.asarray(b2, np.float32).reshape(1, -1),
                "b3r": np.asarray(b3, np.float32).reshape(1, -1),
                "wcm": np.asarray(Wc, np.float32)[: prep["D"]],
                "wcx": np.asarray(Wc, np.float32)[prep["D"] :],
                "bc2": np.asarray(bc, np.float32).reshape(2, 1),
                "icnt": pc["icnt"],
            }
        )
    return maps


# --------------------------------------------------------------------------
# entry point
# --------------------------------------------------------------------------
def kernel(x, edge_index, batch, W1, b1, W2, b2, W3, b3, Wc, bc):
    global LAST_RESULTS
    from concourse import bass_utils

    n_graphs = 512
    shared = os.environ.get("GCN_SHARED", "0") == "1"
    prep = _host_prep(
        x, edge_index, batch, N_CORES, n_graphs, N_CHUNKS, shared_tables=shared
    )
    weights = (W1, b1, W2, b2, W3, b3, Wc, bc)
    nc = _build(prep, weights, TABLE_DT)
    nc.compile()
    maps = _in_maps(prep, weights, TABLE_DT)
    res = bass_utils.run_bass_kernel_spmd(
        nc,
        maps,
        core_ids=list(range(N_CORES)),
        trace=os.environ.get("GCN_TRACE") == "1",
    )
    LAST_RESULTS = res
    outs = [res.results[c]["out"] for c in range(N_CORES)]
    return np.concatenate([np.asarray(o, np.float32).T for o in outs], 0)

